# revision 48
# baseline (speedup 1.0000x reference)
"""Causal self-attention Trainium2 kernel (B=2, T=2048, C=1024, H=16).

Sharding: tensor-parallel over heads (4-way) x data-parallel over batch (2-way)
= 8 cores. Core c handles batch b = c//4 and heads [4*(c%4), 4*(c%4)+4).
Each core computes x @ W_attn for its head slice, causal attention for its 4
heads, and a partial y @ W_proj over its 256 channels. The host sums the 4
partials per batch element (no device collectives).

Matmul operands are fp16 (full-rate PE; fp32 matmul is 4x slower). All PSUM
accumulation is fp32. Weights are host-cast to fp16; activations are cast at
the PSUM->SBUF copy that follows each producing matmul.

Layouts (per core, b fixed):
  xT   [c, t]   : 8 c-tiles of [128, 2048]  (DMA-xbar transposed from x)
  qT/kT[d', t]  : per head-pair hp, [128, 2048]; partitions 0-63 = head 2hp,
                  64-127 = head 2hp+1
  v    [t, d']  : [128 (t in s-tile), 16 s-tiles, 256 (4 local heads x 64)]
  S^T  [s, t]   : scores transposed; softmax sum over s via concurrent
                  ones-column matmuls; no max-subtraction (|S| <~ 3).
  y2 PSUM       : bank0 = y'_a (parts 0-63) + y'_b (parts 64-127),
                  bank1 = denom_a (part 0) + denom_b (part 64)
"""

import sys

if "/opt/trn_rl_repo" not in sys.path:
    sys.path.insert(0, "/opt/trn_rl_repo")

import numpy as np

import concourse.bass as bass
import concourse.bacc as bacc
import concourse.mybir as mybir
import concourse.tile as tile
from concourse.bass_utils import run_bass_kernel_spmd

F32 = mybir.dt.float32
F16 = mybir.dt.float16

B, T, C = 2, 2048, 1024
NH = 16              # total heads
D = 64               # head dim
N_CORES = 8
HG = 4               # heads per core
FC = HG * D          # 256 f-columns per core per q/k/v
CT = C // 128        # 8 c-tiles
TT = T // 128        # 16 t-tiles / s-tiles
TB = T // 512        # 4 t-blocks
NEG = -1.0e10
SCALE = 1.0 / 8.0    # 1/sqrt(D)


def build():
    nc = bacc.Bacc("TRN2", target_bir_lowering=False, debug=False,
                   num_devices=N_CORES)
    x_d = nc.dram_tensor("x", [T, C], F16, kind="ExternalInput").ap()
    wq_d = nc.dram_tensor("wq", [C, FC], F16, kind="ExternalInput").ap()
    wk_d = nc.dram_tensor("wk", [C, FC], F16, kind="ExternalInput").ap()
    wv_d = nc.dram_tensor("wv", [C, FC], F16, kind="ExternalInput").ap()
    wp_d = nc.dram_tensor("wp", [FC, C], F16, kind="ExternalInput").ap()
    out_d = nc.dram_tensor("out", [T, C], F32, kind="ExternalOutput").ap()

    with tile.TileContext(nc) as tc:
        body(tc, x_d, wq_d, wk_d, wv_d, wp_d, out_d)
    nc.compile()
    return nc


def body(tc, x_d, wq_d, wk_d, wv_d, wp_d, out_d):
    nc = tc.nc
    Exp = mybir.ActivationFunctionType.Exp

    with (
        tc.tile_pool(name="sb", bufs=1) as sb,
        tc.tile_pool(name="ps", bufs=1, space="PSUM") as ps,
    ):
        # binary causal mask in S^T orientation: 1 where t - s >= 0 else 0
        mask = sb.tile([128, 128], F16)
        nc.gpsimd.memset(mask, 1.0)
        nc.gpsimd.affine_select(
            out=mask, in_=mask, compare_op=mybir.AluOpType.is_ge,
            fill=0.0, base=0, pattern=[[1, 128]], channel_multiplier=-1)
        ones = sb.tile([128, 64], F16)
        nc.gpsimd.memset(ones, 1.0)


        wq_sb = sb.tile([128, CT, FC], F16)
        wk_sb = sb.tile([128, CT, FC], F16)
        wv_sb = sb.tile([128, CT, FC], F16)
        wp_sb = sb.tile([128, 2, C], F16)          # [c'(128), hp, n]
        nc.sync.dma_start(wv_sb, wv_d.rearrange("(ct p) f -> p ct f", p=128))
        xT = sb.tile([128, CT, T], F16)            # [c_local, ct, t]
        qT = sb.tile([128, 2, T], F16)             # [d', hp, t]
        kT = sb.tile([128, 2, T], F16)
        vp = sb.tile([128, TT, FC], F16)           # [t_in_tile, s_tile, lh*64]
        yT = sb.tile([128, 2, T], F16)             # [d', hp, t]

        # transpose x into xT with the DMA xbar (fp16, 2-byte dtype),
        # one [512, 128] -> [128, 512] transpose per (t-block, c-tile).
        # q/k weights load between t-block 0 (which gates the v matmuls)
        # and the rest of the stream.
        for ci in range(CT):
            nc.sync.dma_start(
                xT[:, ci, 0:512], x_d[0:512, ci * 128:(ci + 1) * 128],
                transpose=True)
        nc.sync.dma_start(wq_sb, wq_d.rearrange("(ct p) f -> p ct f", p=128))
        nc.sync.dma_start(wk_sb, wk_d.rearrange("(ct p) f -> p ct f", p=128))
        for tb in range(1, TB):
            for ci in range(CT):
                nc.sync.dma_start(
                    xT[:, ci, tb * 512:(tb + 1) * 512],
                    x_d[tb * 512:(tb + 1) * 512, ci * 128:(ci + 1) * 128],
                    transpose=True)
        # proj weights are only needed at the end; load them after the
        # latency-critical x transpose stream
        nc.sync.dma_start(wp_sb, wp_d.rearrange("(hp p) n -> p hp n", p=128))

        # Everything except the y' accumulator shares one 3-slot
        # [128,1024] PSUM rotation, and work is emitted "streamed": each
        # attention t-block right after the qkv groups it depends on, so
        # the ACT-bound exp pipeline overlaps qkv's PE-bound matmuls.
        def sp_tile(shape, name, dtype=F32):
            return ps.tile(shape, dtype, tag="sp", name=name, bufs=3)

        # HAM warmup: the PE is DMA-blocked for ~7us at startup, so the
        # first real matmuls would run at the throttled 1.2 GHz clock.
        # ~5us of dummy matmuls on constants (dependency-free) un-throttle
        # the array while the x transposes stream in.
        warm = sp_tile([64, 128], "warm")
        for _ in range(32):
            nc.tensor.matmul(warm, lhsT=ones, rhs=mask, start=True,
                             stop=True)
        # prefetch the exp spline-table load (~2.7us) into the startup
        # window too, so the first real exp doesn't pay it
        warm_e = sb.tile([1, 64], F16)
        nc.scalar.activation(warm_e, ones[0:1, :], Exp, scale=SCALE)

        def emit_v(tt):
            v_ps = sp_tile([128, FC], "v_ps")
            for ci in range(CT):
                nc.tensor.matmul(
                    v_ps,
                    lhsT=xT[:, ci, tt * 128:(tt + 1) * 128],
                    rhs=wv_sb[:, ci, :],
                    start=(ci == 0), stop=(ci == CT - 1))
            nc.vector.tensor_copy(vp[:, tt, :], v_ps)

        def emit_qk_group(hp, w_sb, dst, tb):
            qk_ps = sp_tile([128, 512], "qk_ps")
            for ci in range(CT):
                nc.tensor.matmul(
                    qk_ps,
                    lhsT=w_sb[:, ci, hp * 128:(hp + 1) * 128],
                    rhs=xT[:, ci, tb * 512:(tb + 1) * 512],
                    start=(ci == 0), stop=(ci == CT - 1))
            nc.vector.tensor_copy(
                dst[:, hp, tb * 512:(tb + 1) * 512], qk_ps)

        def emit_att_tb(hp, tb):
            lha, lhb = 2 * hp, 2 * hp + 1
            n_st = 4 * tb + 4              # causal: s-tiles 0..4tb+3
            # y2 bank0: y' both heads; bank1: denominators
            y2 = ps.tile([128, 1024], F32, tag="y2", bufs=1)
            for pair in range(n_st // 2):
                pts = []
                for hi in (0, 1):
                    sp = sp_tile([128, 1024], f"sp{hi}")
                    u0 = 0
                    for q2 in (0, 1):
                        si = 2 * pair + q2
                        kd = si - 4 * tb
                        col0 = 128 * kd if kd >= 0 else 0
                        if q2 == 0:
                            u0 = col0
                        nc.tensor.matmul(
                            sp[:, q2 * 512 + col0:(q2 + 1) * 512],
                            lhsT=kT[64 * hi:64 * hi + 64, hp,
                                    si * 128:(si + 1) * 128],
                            rhs=qT[64 * hi:64 * hi + 64, hp,
                                   tb * 512 + col0:(tb + 1) * 512],
                            start=True, stop=True)
                    pt = sb.tile([128, 1024], F16, tag=f"pt{hi}",
                                 name=f"pt{hi}", bufs=3)
                    nc.scalar.activation(pt[:, u0:], sp[:, u0:], Exp,
                                         scale=SCALE)
                    # zero the invalid triangle of diagonal squares after
                    # exp (exp * 0 == masked exp, and it keeps the DVE off
                    # the S -> exp critical path)
                    for q2 in (0, 1):
                        si = 2 * pair + q2
                        kd = si - 4 * tb
                        if kd >= 0:
                            col0 = 128 * kd
                            psl = pt[:, q2 * 512 + col0:
                                     q2 * 512 + col0 + 128]
                            nc.vector.tensor_mul(psl, psl, mask)
                    pts.append(pt)
                # PV + denominator accumulation; on the final s-tile the
                # denominators go first so the reciprocals overlap the
                # last V matmuls
                for q2 in (0, 1):
                    si = 2 * pair + q2
                    kd = si - 4 * tb
                    col0 = 128 * kd if kd >= 0 else 0
                    first, last = si == 0, si == n_st - 1
                    pa, pb = pts
                    vmm = [
                        (y2[0:64, col0:512],
                         vp[:, si, 64 * lha:64 * lha + 64], pa),
                        (y2[64:128, col0:512],
                         vp[:, si, 64 * lhb:64 * lhb + 64], pb),
                    ]
                    dmm = [
                        (y2[0:1, 512 + col0:1024], ones[:, 0:1], pa),
                        (y2[64:65, 512 + col0:1024], ones[:, 0:1], pb),
                    ]
                    groups = dmm + vmm if last else vmm + dmm
                    for out_ap, w_ap, p_ap in groups:
                        nc.tensor.matmul(
                            out_ap, lhsT=w_ap,
                            rhs=p_ap[:, q2 * 512 + col0:(q2 + 1) * 512],
                            start=first, stop=last)
            # normalize: yT = y' * (1/denom) broadcast across partitions
            rcp = sb.tile([128, 512], F16, tag="rcp", bufs=3)
            with nc.allow_low_precision(reason="softmax denom recip f16"):
                nc.vector.reciprocal(rcp[0:1, :], y2[0:1, 512:1024])
                nc.vector.reciprocal(rcp[64:65, :], y2[64:65, 512:1024])
            # the denominator bank of y2 is dead after the reciprocals;
            # broadcast into it instead of taking a slot from the rotation
            nc.tensor.matmul(y2[0:64, 512:1024], lhsT=ones[0:1, :],
                             rhs=rcp[0:1, :], start=True, stop=True)
            nc.tensor.matmul(y2[64:128, 512:1024], lhsT=ones[64:65, :],
                             rhs=rcp[64:65, :], start=True, stop=True)
            bp_sb = sb.tile([128, 512], F32, tag="bps", bufs=3)
            nc.vector.tensor_copy(bp_sb, y2[:, 512:1024])
            nc.vector.tensor_mul(
                yT[0:64, hp, tb * 512:(tb + 1) * 512],
                y2[0:64, 0:512], bp_sb[0:64, :])
            nc.vector.tensor_mul(
                yT[64:128, hp, tb * 512:(tb + 1) * 512],
                y2[64:128, 0:512], bp_sb[64:128, :])

        def emit_proj(tt):
            pj = sp_tile([128, 1024], "pj")
            for hp in range(2):
                for nb in range(2):
                    nc.tensor.matmul(
                        pj[:, nb * 512:(nb + 1) * 512],
                        lhsT=yT[:, hp, tt * 128:(tt + 1) * 128],
                        rhs=wp_sb[:, hp, nb * 512:(nb + 1) * 512],
                        start=(hp == 0), stop=(hp == 1))
            ob = sb.tile([128, 1024], F32, tag="ob", bufs=4)
            nc.scalar.copy(ob[:, 0:512], pj[:, 0:512])
            nc.vector.tensor_copy(ob[:, 512:1024], pj[:, 512:1024])
            nc.sync.dma_start(out_d[tt * 128:(tt + 1) * 128, :], ob)

        # ---- streamed emission ----
        for tb in range(TB):
            for tt in range(4 * tb, 4 * tb + 4):
                emit_v(tt)
            emit_qk_group(0, wq_sb, qT, tb)
            emit_qk_group(0, wk_sb, kT, tb)
            emit_att_tb(0, tb)
        # hp1's first q/k groups fill hp0's final normalize tail
        emit_qk_group(1, wq_sb, qT, 0)
        emit_qk_group(1, wk_sb, kT, 0)
        for tb in range(TB):
            emit_att_tb(1, tb)
            # prefetch the next t-block's q/k before this block's proj so
            # the next attention block starts without waiting behind proj
            if tb + 1 < TB:
                emit_qk_group(1, wq_sb, qT, tb + 1)
                emit_qk_group(1, wk_sb, kT, tb + 1)
            for tt in range(4 * tb, 4 * tb + 4):
                emit_proj(tt)


_NC_CACHE = None


def _get_nc():
    global _NC_CACHE
    if _NC_CACHE is None:
        _NC_CACHE = build()
    return _NC_CACHE


def _in_maps(x, W_attn, W_proj):
    x16 = x.astype(np.float16)
    wa16 = W_attn.astype(np.float16)
    wp16 = W_proj.astype(np.float16)
    maps = []
    for core in range(N_CORES):
        b, g = core // 4, core % 4
        f0 = FC * g
        maps.append({
            "x": np.ascontiguousarray(x16[b]),
            "wq": np.ascontiguousarray(wa16[:, f0:f0 + FC]),
            "wk": np.ascontiguousarray(wa16[:, C + f0:C + f0 + FC]),
            "wv": np.ascontiguousarray(wa16[:, 2 * C + f0:2 * C + f0 + FC]),
            "wp": np.ascontiguousarray(wp16[f0:f0 + FC, :]),
        })
    return maps


def run(x, W_attn, W_proj, trace=False, **kwargs):
    nc = _get_nc()
    res = run_bass_kernel_spmd(nc, _in_maps(x, W_attn, W_proj),
                               core_ids=list(range(N_CORES)),
                               trace=trace, **kwargs)
    out = np.zeros((B, T, C), dtype=np.float32)
    for core in range(N_CORES):
        out[core // 4] += res.results[core]["out"]
    return out, res


def kernel(x, W_attn, W_proj):
    x = np.asarray(x, dtype=np.float32)
    W_attn = np.asarray(W_attn, dtype=np.float32)
    W_proj = np.asarray(W_proj, dtype=np.float32)
    out, _ = run(x, W_attn, W_proj, trace=False)
    return out


# revision 49
# speedup vs baseline: 1.0013x; 1.0013x over previous
"""Causal self-attention Trainium2 kernel (B=2, T=2048, C=1024, H=16).

Sharding: tensor-parallel over heads (4-way) x data-parallel over batch (2-way)
= 8 cores. Core c handles batch b = c//4 and heads [4*(c%4), 4*(c%4)+4).
Each core computes x @ W_attn for its head slice, causal attention for its 4
heads, and a partial y @ W_proj over its 256 channels. The host sums the 4
partials per batch element (no device collectives).

Matmul operands are fp16 (full-rate PE; fp32 matmul is 4x slower). All PSUM
accumulation is fp32. Weights are host-cast to fp16; activations are cast at
the PSUM->SBUF copy that follows each producing matmul.

Layouts (per core, b fixed):
  xT   [c, t]   : 8 c-tiles of [128, 2048]  (DMA-xbar transposed from x)
  qT/kT[d', t]  : per head-pair hp, [128, 2048]; partitions 0-63 = head 2hp,
                  64-127 = head 2hp+1
  v    [t, d']  : [128 (t in s-tile), 16 s-tiles, 256 (4 local heads x 64)]
  S^T  [s, t]   : scores transposed; softmax sum over s via concurrent
                  ones-column matmuls; no max-subtraction (|S| <~ 3).
  y2 PSUM       : bank0 = y'_a (parts 0-63) + y'_b (parts 64-127),
                  bank1 = denom_a (part 0) + denom_b (part 64)
"""

import sys

if "/opt/trn_rl_repo" not in sys.path:
    sys.path.insert(0, "/opt/trn_rl_repo")

import numpy as np

import concourse.bass as bass
import concourse.bacc as bacc
import concourse.mybir as mybir
import concourse.tile as tile
from concourse.bass_utils import run_bass_kernel_spmd

F32 = mybir.dt.float32
F16 = mybir.dt.float16

B, T, C = 2, 2048, 1024
NH = 16              # total heads
D = 64               # head dim
N_CORES = 8
HG = 4               # heads per core
FC = HG * D          # 256 f-columns per core per q/k/v
CT = C // 128        # 8 c-tiles
TT = T // 128        # 16 t-tiles / s-tiles
TB = T // 512        # 4 t-blocks
NEG = -1.0e10
SCALE = 1.0 / 8.0    # 1/sqrt(D)


def build():
    nc = bacc.Bacc("TRN2", target_bir_lowering=False, debug=False,
                   num_devices=N_CORES)
    x_d = nc.dram_tensor("x", [T, C], F16, kind="ExternalInput").ap()
    wq_d = nc.dram_tensor("wq", [C, FC], F16, kind="ExternalInput").ap()
    wk_d = nc.dram_tensor("wk", [C, FC], F16, kind="ExternalInput").ap()
    wv_d = nc.dram_tensor("wv", [C, FC], F16, kind="ExternalInput").ap()
    wp_d = nc.dram_tensor("wp", [FC, C], F16, kind="ExternalInput").ap()
    out_d = nc.dram_tensor("out", [T, C], F32, kind="ExternalOutput").ap()

    with tile.TileContext(nc) as tc:
        body(tc, x_d, wq_d, wk_d, wv_d, wp_d, out_d)
    nc.compile()
    return nc


def body(tc, x_d, wq_d, wk_d, wv_d, wp_d, out_d):
    nc = tc.nc
    Exp = mybir.ActivationFunctionType.Exp

    with (
        tc.tile_pool(name="sb", bufs=1) as sb,
        tc.tile_pool(name="ps", bufs=1, space="PSUM") as ps,
    ):
        # binary causal mask in S^T orientation: 1 where t - s >= 0 else 0
        mask = sb.tile([128, 128], F16)
        nc.gpsimd.memset(mask, 1.0)
        nc.gpsimd.affine_select(
            out=mask, in_=mask, compare_op=mybir.AluOpType.is_ge,
            fill=0.0, base=0, pattern=[[1, 128]], channel_multiplier=-1)
        ones = sb.tile([128, 64], F16)
        nc.gpsimd.memset(ones, 1.0)


        wq_sb = sb.tile([128, CT, FC], F16)
        wk_sb = sb.tile([128, CT, FC], F16)
        wv_sb = sb.tile([128, CT, FC], F16)
        wp_sb = sb.tile([128, 2, C], F16)          # [c'(128), hp, n]
        nc.sync.dma_start(wv_sb, wv_d.rearrange("(ct p) f -> p ct f", p=128))
        xT = sb.tile([128, CT, T], F16)            # [c_local, ct, t]
        qT = sb.tile([128, 2, T], F16)             # [d', hp, t]
        kT = sb.tile([128, 2, T], F16)
        vp = sb.tile([128, TT, FC], F16)           # [t_in_tile, s_tile, lh*64]
        yT = sb.tile([128, 2, T], F16)             # [d', hp, t]

        # transpose x into xT with the DMA xbar (fp16, 2-byte dtype),
        # one [512, 128] -> [128, 512] transpose per (t-block, c-tile).
        # q/k weights load between t-block 0 (which gates the v matmuls)
        # and the rest of the stream.
        nc.sync.dma_start(wq_sb, wq_d.rearrange("(ct p) f -> p ct f", p=128))
        for ci in range(CT):
            nc.sync.dma_start(
                xT[:, ci, 0:512], x_d[0:512, ci * 128:(ci + 1) * 128],
                transpose=True)
        nc.sync.dma_start(wk_sb, wk_d.rearrange("(ct p) f -> p ct f", p=128))
        for tb in range(1, TB):
            for ci in range(CT):
                nc.sync.dma_start(
                    xT[:, ci, tb * 512:(tb + 1) * 512],
                    x_d[tb * 512:(tb + 1) * 512, ci * 128:(ci + 1) * 128],
                    transpose=True)
        # proj weights are only needed at the end; load them after the
        # latency-critical x transpose stream
        nc.sync.dma_start(wp_sb, wp_d.rearrange("(hp p) n -> p hp n", p=128))

        # Everything except the y' accumulator shares one 3-slot
        # [128,1024] PSUM rotation, and work is emitted "streamed": each
        # attention t-block right after the qkv groups it depends on, so
        # the ACT-bound exp pipeline overlaps qkv's PE-bound matmuls.
        def sp_tile(shape, name, dtype=F32):
            return ps.tile(shape, dtype, tag="sp", name=name, bufs=3)

        # HAM warmup: the PE is DMA-blocked for ~7us at startup, so the
        # first real matmuls would run at the throttled 1.2 GHz clock.
        # ~5us of dummy matmuls on constants (dependency-free) un-throttle
        # the array while the x transposes stream in.
        warm = sp_tile([64, 128], "warm")
        for _ in range(32):
            nc.tensor.matmul(warm, lhsT=ones, rhs=mask, start=True,
                             stop=True)
        # prefetch the exp spline-table load (~2.7us) into the startup
        # window too, so the first real exp doesn't pay it
        warm_e = sb.tile([1, 64], F16)
        nc.scalar.activation(warm_e, ones[0:1, :], Exp, scale=SCALE)

        def emit_v(tt):
            v_ps = sp_tile([128, FC], "v_ps")
            for ci in range(CT):
                nc.tensor.matmul(
                    v_ps,
                    lhsT=xT[:, ci, tt * 128:(tt + 1) * 128],
                    rhs=wv_sb[:, ci, :],
                    start=(ci == 0), stop=(ci == CT - 1))
            nc.vector.tensor_copy(vp[:, tt, :], v_ps)

        def emit_qk_group(hp, w_sb, dst, tb):
            qk_ps = sp_tile([128, 512], "qk_ps")
            for ci in range(CT):
                nc.tensor.matmul(
                    qk_ps,
                    lhsT=w_sb[:, ci, hp * 128:(hp + 1) * 128],
                    rhs=xT[:, ci, tb * 512:(tb + 1) * 512],
                    start=(ci == 0), stop=(ci == CT - 1))
            nc.vector.tensor_copy(
                dst[:, hp, tb * 512:(tb + 1) * 512], qk_ps)

        def emit_att_tb(hp, tb):
            lha, lhb = 2 * hp, 2 * hp + 1
            n_st = 4 * tb + 4              # causal: s-tiles 0..4tb+3
            # y2 bank0: y' both heads; bank1: denominators
            y2 = ps.tile([128, 1024], F32, tag="y2", bufs=1)
            for pair in range(n_st // 2):
                pts = []
                for hi in (0, 1):
                    sp = sp_tile([128, 1024], f"sp{hi}")
                    u0 = 0
                    for q2 in (0, 1):
                        si = 2 * pair + q2
                        kd = si - 4 * tb
                        col0 = 128 * kd if kd >= 0 else 0
                        if q2 == 0:
                            u0 = col0
                        nc.tensor.matmul(
                            sp[:, q2 * 512 + col0:(q2 + 1) * 512],
                            lhsT=kT[64 * hi:64 * hi + 64, hp,
                                    si * 128:(si + 1) * 128],
                            rhs=qT[64 * hi:64 * hi + 64, hp,
                                   tb * 512 + col0:(tb + 1) * 512],
                            start=True, stop=True)
                    pt = sb.tile([128, 1024], F16, tag=f"pt{hi}",
                                 name=f"pt{hi}", bufs=3)
                    nc.scalar.activation(pt[:, u0:], sp[:, u0:], Exp,
                                         scale=SCALE)
                    # zero the invalid triangle of diagonal squares after
                    # exp (exp * 0 == masked exp, and it keeps the DVE off
                    # the S -> exp critical path)
                    for q2 in (0, 1):
                        si = 2 * pair + q2
                        kd = si - 4 * tb
                        if kd >= 0:
                            col0 = 128 * kd
                            psl = pt[:, q2 * 512 + col0:
                                     q2 * 512 + col0 + 128]
                            nc.vector.tensor_mul(psl, psl, mask)
                    pts.append(pt)
                # PV + denominator accumulation; on the final s-tile the
                # denominators go first so the reciprocals overlap the
                # last V matmuls
                for q2 in (0, 1):
                    si = 2 * pair + q2
                    kd = si - 4 * tb
                    col0 = 128 * kd if kd >= 0 else 0
                    first, last = si == 0, si == n_st - 1
                    pa, pb = pts
                    vmm = [
                        (y2[0:64, col0:512],
                         vp[:, si, 64 * lha:64 * lha + 64], pa),
                        (y2[64:128, col0:512],
                         vp[:, si, 64 * lhb:64 * lhb + 64], pb),
                    ]
                    dmm = [
                        (y2[0:1, 512 + col0:1024], ones[:, 0:1], pa),
                        (y2[64:65, 512 + col0:1024], ones[:, 0:1], pb),
                    ]
                    groups = dmm + vmm if last else vmm + dmm
                    for out_ap, w_ap, p_ap in groups:
                        nc.tensor.matmul(
                            out_ap, lhsT=w_ap,
                            rhs=p_ap[:, q2 * 512 + col0:(q2 + 1) * 512],
                            start=first, stop=last)
            # normalize: yT = y' * (1/denom) broadcast across partitions
            rcp = sb.tile([128, 512], F16, tag="rcp", bufs=3)
            with nc.allow_low_precision(reason="softmax denom recip f16"):
                nc.vector.reciprocal(rcp[0:1, :], y2[0:1, 512:1024])
                nc.vector.reciprocal(rcp[64:65, :], y2[64:65, 512:1024])
            # the denominator bank of y2 is dead after the reciprocals;
            # broadcast into it instead of taking a slot from the rotation
            nc.tensor.matmul(y2[0:64, 512:1024], lhsT=ones[0:1, :],
                             rhs=rcp[0:1, :], start=True, stop=True)
            nc.tensor.matmul(y2[64:128, 512:1024], lhsT=ones[64:65, :],
                             rhs=rcp[64:65, :], start=True, stop=True)
            bp_sb = sb.tile([128, 512], F32, tag="bps", bufs=3)
            nc.vector.tensor_copy(bp_sb, y2[:, 512:1024])
            nc.vector.tensor_mul(
                yT[0:64, hp, tb * 512:(tb + 1) * 512],
                y2[0:64, 0:512], bp_sb[0:64, :])
            nc.vector.tensor_mul(
                yT[64:128, hp, tb * 512:(tb + 1) * 512],
                y2[64:128, 0:512], bp_sb[64:128, :])

        def emit_proj(tt):
            pj = sp_tile([128, 1024], "pj")
            for hp in range(2):
                for nb in range(2):
                    nc.tensor.matmul(
                        pj[:, nb * 512:(nb + 1) * 512],
                        lhsT=yT[:, hp, tt * 128:(tt + 1) * 128],
                        rhs=wp_sb[:, hp, nb * 512:(nb + 1) * 512],
                        start=(hp == 0), stop=(hp == 1))
            ob = sb.tile([128, 1024], F32, tag="ob", bufs=4)
            nc.scalar.copy(ob[:, 0:512], pj[:, 0:512])
            nc.vector.tensor_copy(ob[:, 512:1024], pj[:, 512:1024])
            nc.sync.dma_start(out_d[tt * 128:(tt + 1) * 128, :], ob)

        # ---- streamed emission ----
        for tb in range(TB):
            for tt in range(4 * tb, 4 * tb + 4):
                emit_v(tt)
            emit_qk_group(0, wq_sb, qT, tb)
            emit_qk_group(0, wk_sb, kT, tb)
            emit_att_tb(0, tb)
        # hp1's first q/k groups fill hp0's final normalize tail
        emit_qk_group(1, wq_sb, qT, 0)
        emit_qk_group(1, wk_sb, kT, 0)
        for tb in range(TB):
            emit_att_tb(1, tb)
            # prefetch the next t-block's q/k before this block's proj so
            # the next attention block starts without waiting behind proj
            if tb + 1 < TB:
                emit_qk_group(1, wq_sb, qT, tb + 1)
                emit_qk_group(1, wk_sb, kT, tb + 1)
            for tt in range(4 * tb, 4 * tb + 4):
                emit_proj(tt)


_NC_CACHE = None


def _get_nc():
    global _NC_CACHE
    if _NC_CACHE is None:
        _NC_CACHE = build()
    return _NC_CACHE


def _in_maps(x, W_attn, W_proj):
    x16 = x.astype(np.float16)
    wa16 = W_attn.astype(np.float16)
    wp16 = W_proj.astype(np.float16)
    maps = []
    for core in range(N_CORES):
        b, g = core // 4, core % 4
        f0 = FC * g
        maps.append({
            "x": np.ascontiguousarray(x16[b]),
            "wq": np.ascontiguousarray(wa16[:, f0:f0 + FC]),
            "wk": np.ascontiguousarray(wa16[:, C + f0:C + f0 + FC]),
            "wv": np.ascontiguousarray(wa16[:, 2 * C + f0:2 * C + f0 + FC]),
            "wp": np.ascontiguousarray(wp16[f0:f0 + FC, :]),
        })
    return maps


def run(x, W_attn, W_proj, trace=False, **kwargs):
    nc = _get_nc()
    res = run_bass_kernel_spmd(nc, _in_maps(x, W_attn, W_proj),
                               core_ids=list(range(N_CORES)),
                               trace=trace, **kwargs)
    out = np.zeros((B, T, C), dtype=np.float32)
    for core in range(N_CORES):
        out[core // 4] += res.results[core]["out"]
    return out, res


def kernel(x, W_attn, W_proj):
    x = np.asarray(x, dtype=np.float32)
    W_attn = np.asarray(W_attn, dtype=np.float32)
    W_proj = np.asarray(W_proj, dtype=np.float32)
    out, _ = run(x, W_attn, W_proj, trace=False)
    return out
